# revision 3
# baseline (speedup 1.0000x reference)
"""BitExpert (BitNet-style MLP) Trainium2 kernel v2, 8-core data-parallel.

y = bitlinear(silu(bitlinear(x,w1)) * bitlinear(x,w3), w2)
  with per-token int8 activation quant and per-tensor ternary weight quant.

Strategy (8 NeuronCores, SPMD single NEFF), data-parallel over tokens:
each core takes 1024 of 8192 token rows and a full weight copy.

v2 changes vs v1 (1.495 ms):
 - Redundant-LDWEIGHTS dedup pass: consecutive matmuls sharing one
   stationary operand skip the PE weight reload, so fill overlaps drain
   (measured 267.7 -> ~220 ns per N=512 bf16 matmul).
 - Phase 1 emits w1/w3 matmuls as pairs sharing the xq stationary chunk.
 - Phase 3 pairs two d_model 512-blocks per hq stationary chunk
   (w2 ternarized row-major in column-pair passes).
 - hqT gets its own buffer (no xqT alias): h quantization overlaps the
   last phase-1 block and phase 3 starts immediately after. The 88 KB
   hqT region doubles as prologue DMA staging (disjoint lifetimes).
 - Prologue: wsl partial sums -> AllReduce issued before x quant work.
 - Eviction fuses h=silu*h3 multiply with the per-token absmax update
   via tensor_tensor_reduce(abs_max).
 - Integer arithmetic exactness preserved throughout (bf16 holds ints
   in [-128,127] and ternary weights exactly; f32 PSUM accumulation of
   integer dot products < 2^24 is exact).
"""
import numpy as np

import concourse.mybir as mybir
import concourse.tile as tile
from concourse import bass_utils, bacc

F32 = mybir.dt.float32
BF16 = mybir.dt.bfloat16
AX = mybir.AxisListType
OP = mybir.AluOpType
ACTF = mybir.ActivationFunctionType

NCORES = 8
D = 2048           # d_model
H = 5632           # hidden
TOK = 8192         # total tokens
T = TOK // NCORES  # tokens per core (1024)
P = 128
TT = T // P        # token tiles per core (8)
HB = 512           # hidden block (phase 1)
NHB = H // HB      # 11
KD = D // P        # 16
KH = H // P        # 44
XC = 1024          # x load chunk
GC = 512           # h/g quant chunk width
NGC = H // GC      # 11
WSLC = 2816        # wsl partial-sum chunk width
NWC = 11264 // WSLC  # 4

MAGIC = 12582912.0             # 1.5 * 2^23
EPS = 1e-5
NW = float(H * D)
WSLF = (H * D) // NCORES // P  # 11264
RECIP_NW = float(np.float32(1.0) / np.float32(NW))


def dedup_ldweights(nc):
    """Drop InstLdweights that reload the identical stationary operand.

    Consecutive matmuls sharing one stationary then overlap fill/drain on
    the PE array (no weight swap between them). Only LDWs with no sync
    waits/updates are dropped; any other PE instruction resets tracking.
    Tile generations are distinguished by memref (unique tile names).
    """
    removed = 0
    for f in nc.m.functions:
        for b in f.blocks:
            insts = b.instructions
            last_key = None
            drop = []
            for idx, i in enumerate(insts):
                tn = type(i).__name__
                if tn == 'InstLdweights':
                    if getattr(i, 'is_transpose', None):
                        last_key = None
                        continue
                    p = i.ins[0]
                    key = (p.memref, p.offset, str(p.ap), str(p.dtype),
                           str(getattr(i, 'perf_mode', None)))
                    si = i.sync_info
                    no_sync = si is None or (not si.on_wait and not si.on_update)
                    if key == last_key and no_sync:
                        drop.append(idx)
                        removed += 1
                        continue
                    last_key = key
                elif tn == 'InstMatmult':
                    if getattr(i, 'is_transpose', None):
                        last_key = None
                elif str(getattr(i, 'engine', '')) == 'EngineType.PE':
                    last_key = None
            for idx in reversed(drop):
                del insts[idx]
    return removed


def _build(use_collective=True):
    nc = bacc.Bacc("TRN2", target_bir_lowering=False, debug=False,
                   num_devices=NCORES if use_collective else 1)
    x = nc.dram_tensor("x", [T, D], F32, kind="ExternalInput").ap()
    w1t = nc.dram_tensor("w1t", [D, H], F32, kind="ExternalInput").ap()
    w2t = nc.dram_tensor("w2t", [H, D], F32, kind="ExternalInput").ap()
    w3t = nc.dram_tensor("w3t", [D, H], F32, kind="ExternalInput").ap()
    wsl = nc.dram_tensor("wsl", [3, P, WSLF], F32, kind="ExternalInput").ap()
    y = nc.dram_tensor("y", [T, D], F32, kind="ExternalOutput").ap()

    cc_in = nc.dram_tensor("cc_in", [1, 4], F32)
    cc_out = nc.dram_tensor("cc_out", [1, 4], F32, addr_space="Shared")

    with tile.TileContext(nc) as tc:
        _body(nc, tc, x, w1t, w2t, w3t, wsl, y, cc_in, cc_out,
              use_collective)
    removed = dedup_ldweights(nc)
    print(f"dedup_ldweights removed {removed}")
    nc.compile()
    return nc


def _body(nc, tc, x, w1t, w2t, w3t, wsl, y, cc_in, cc_out,
          use_collective=True):
    ctxs = []

    def pool(name, bufs, space="SBUF"):
        cm = tc.tile_pool(name=name, bufs=bufs, space=space)
        p = cm.__enter__()
        ctxs.append(cm)
        return p

    singles = pool("singles", 1)
    dramp = pool("dram", 1, space="DRAM")
    big = pool("big", 1)       # per-tag rings: xq(2) + wT(4), 16KB slots
    hqp = pool("hqp", 1)       # 88KB: prologue staging, then hqT
    stage = pool("stage", 3)   # [P, 512] f32 DMA staging
    qb = pool("qb", 2)         # 2KB slots: bf16 naturals + y staging
    hwork = pool("hwork", 3)   # [P, 512] f32 eviction tiles
    scal = pool("scal", 5)     # small scratch columns
    yout = qb                  # share slots with qb (disjoint-phase use)
    ps1 = pool("ps1", 3, space="PSUM")   # phase-1 psa/psb [P,512]
    ps3 = pool("ps3", 4, space="PSUM")   # phase-3 pair banks [P,512]

    g_dram = dramp.tile([T, H], F32)

    # persistent per-token scalar columns
    mh_all = singles.tile([P, TT], F32)
    sx_all = singles.tile([P, TT], F32)
    al_all = singles.tile([P, TT], F32)
    be_all = singles.tile([P, TT], F32)
    sh_all = singles.tile([P, TT], F32)
    de_all = singles.tile([P, TT], F32)
    rx_all = singles.tile([P, TT], F32)
    cvec = singles.tile([P, 8], F32)   # [c1 c3 c2 _ s1 s3 s2 _]
    negmagic = singles.tile([P, 1], F32)
    nc.vector.memset(negmagic[:], -MAGIC)
    nc.vector.memset(mh_all[:], 0.0)

    # ---------------- weight mean partials + AllReduce ----------------
    partials = singles.tile([P, 4], F32)
    nc.vector.memset(partials[:], 1.0)  # col 3 dummy (avoids 1/0 later)
    wsl_stage = hqp.tile([P, 2, WSLC], F32, tag="hqp", name="wsl_stage")
    for j in range(3):
        for ch in range(NWC):
            buf = wsl_stage[:, (j * NWC + ch) % 2, :]
            nc.sync.dma_start(buf, wsl[j, :, ch * WSLC:(ch + 1) * WSLC])
            s1 = scal.tile([P, WSLC // P], F32, tag="scal", name="msum1")
            nc.vector.tensor_reduce(
                s1[:], buf.rearrange("p (a b) -> p a b", b=P),
                AX.X, OP.add, apply_absolute_value=True)
            s2 = scal.tile([P, 1], F32, tag="scal", name="msum2")
            nc.vector.tensor_reduce(s2[:], s1[:], AX.X, OP.add)
            if ch == 0:
                nc.vector.tensor_copy(partials[:, j:j + 1], s2[:])
            else:
                nc.vector.tensor_tensor(partials[:, j:j + 1],
                                        partials[:, j:j + 1], s2[:], OP.add)
    ones = singles.tile([P, 1], F32)
    nc.vector.memset(ones[:], 1.0)
    psums = ps1.tile([1, 4], F32, tag="ps")
    nc.tensor.matmul(psums[:], ones[:], partials[:], start=True, stop=True)
    row4 = singles.tile([1, 4], F32)
    nc.vector.tensor_copy(row4[:], psums[:])
    nc.sync.dma_start(cc_in.ap(), row4[:])
    if use_collective:
        nc.gpsimd.collective_compute(
            "AllReduce", OP.add, replica_groups=[list(range(NCORES))],
            ins=[cc_in.ap()], outs=[cc_out.ap()])
    else:
        nc.sync.dma_start(cc_out.ap(), row4[:])
    sums = singles.tile([1, 4], F32)
    nc.sync.dma_start(sums[:], cc_out.ap())
    row8 = singles.tile([1, 8], F32)
    nc.vector.tensor_scalar(row8[:, 0:4], sums[:], RECIP_NW, EPS,
                            OP.mult, OP.max)
    nc.vector.reciprocal(row8[:, 4:8], row8[:, 0:4])
    nc.gpsimd.partition_broadcast(cvec[:], row8[:])
    c1, c3, c2 = cvec[:, 0:1], cvec[:, 1:2], cvec[:, 2:3]
    s1c, s3c, s2c = cvec[:, 4:5], cvec[:, 5:6], cvec[:, 6:7]

    # ---------------- x: absmax, quantize, transpose ----------------
    # xq stored as two [P, 8, T] bf16 tiles (16 KB big-pool slots)
    xq = [big.tile([P, KD // 2, T], BF16, tag="xq", bufs=2, name=f"xq{i}")
          for i in range(2)]
    x_stage = hqp.tile([P, 4, XC], F32, tag="hqp", name="x_stage")

    def emit_x_tile(tt):
        xts = []
        mx = scal.tile([P, 1], F32, tag="scal", name="mx")
        for cix in range(D // XC):
            xt = x_stage[:, 2 * (tt % 2) + cix, :]
            nc.sync.dma_start(
                xt, x[tt * P:(tt + 1) * P, cix * XC:(cix + 1) * XC])
            xts.append(xt)
            mc = scal.tile([P, 1], F32, tag="scal", name="mxc")
            nc.vector.tensor_reduce(mc[:], xt, AX.X, OP.max,
                                    apply_absolute_value=True)
            if cix == 0:
                nc.vector.tensor_scalar(mx[:], mc[:], EPS, None, OP.max)
            else:
                nc.vector.tensor_tensor(mx[:], mx[:], mc[:], OP.max)
        rec = scal.tile([P, 1], F32, tag="scal", name="rec")
        nc.vector.reciprocal(rec[:], mx[:])
        sx = sx_all[:, tt:tt + 1]
        nc.vector.tensor_scalar(sx, rec[:], 127.0, None, OP.mult)
        nc.vector.reciprocal(rx_all[:, tt:tt + 1], sx)
        for cix in range(D // XC):
            xt = xts[cix]
            nc.vector.tensor_scalar(xt, xt, sx, MAGIC, OP.mult, OP.add)
            xqn = qb.tile([P, XC], BF16, tag="qb")
            nc.vector.tensor_scalar(xqn[:], xt, MAGIC, None, OP.subtract)
            nc.sync.dma_start_transpose(
                xq[cix][:, :, tt * P:(tt + 1) * P], xqn[:])

    for tt in range(TT):
        emit_x_tile(tt)
    nc.vector.tensor_tensor(al_all[:], rx_all[:], c1.to_broadcast((P, TT)),
                            OP.mult)
    nc.vector.tensor_tensor(be_all[:], rx_all[:], c3.to_broadcast((P, TT)),
                            OP.mult)

    # hqT: quantized h, hidden-on-partitions (88 KB; reuses staging slot)
    hqT = hqp.tile([P, KH, T], BF16, tag="hqp", name="hqT")

    # ---------------- ternarize helpers ----------------
    def tern_to(dst_ap, src_dram_ap, scol):
        """One [P, 512] f32 chunk -> ternary bf16 into dst_ap."""
        src = stage.tile([P, 512], F32, tag="stage", name="wsrc")
        nc.sync.dma_start(src[:], src_dram_ap)
        nc.vector.tensor_scalar(src[:], src[:], scol, 1.49, OP.mult, OP.min)
        nc.vector.tensor_scalar(src[:], src[:], -1.49, MAGIC, OP.max, OP.add)
        nc.scalar.activation(dst_ap, src[:], ACTF.Identity,
                             bias=negmagic[:, 0:1])

    def build_w13(hb):
        """w1|w3 ternary combined block for hidden block hb.

        Two tiles [P, 8, 1024]; inner row ko: cols 0:512 = w1, 512:1024 = w3.
        """
        tiles = [big.tile([P, 8, 2 * HB], BF16, tag="wT", bufs=4,
                          name=f"wT{hb}_{i}") for i in range(2)]
        for ko in range(KD):
            dst = tiles[ko // 8]
            r = ko % 8
            cols = slice(hb * HB, (hb + 1) * HB)
            rows = slice(ko * P, (ko + 1) * P)
            tern_to(dst[:, r, 0:HB], w1t[rows, cols], s1c)
            tern_to(dst[:, r, HB:2 * HB], w3t[rows, cols], s3c)
        return tiles

    def build_w2_tile(ti, colbase):
        """w2 ternary chunk tile: [P, 8, 1024] covering hc 8ti..8ti+7,
        d_model cols colbase..colbase+1024."""
        tg, nb = ("wT", 4) if ti < 4 else ("xq", 2)
        dst = big.tile([P, 8, 1024], BF16, tag=tg, bufs=nb,
                       name=f"w2c{colbase}_{ti}")
        for r in range(8):
            hc = 8 * ti + r
            if hc >= KH:
                break
            rows = slice(hc * P, (hc + 1) * P)
            tern_to(dst[:, r, 0:512], w2t[rows, colbase:colbase + 512], s2c)
            tern_to(dst[:, r, 512:1024],
                    w2t[rows, colbase + 512:colbase + 1024], s2c)
        return dst

    # ---------------- h quant emission (during last phase-1 block) ----
    def emit_sh(tt):
        cs = slice(tt, tt + 1)
        stmp = scal.tile([P, 1], F32, tag="scal", name="stmp")
        nc.vector.tensor_scalar(stmp[:], mh_all[:, cs], EPS, None, OP.max)
        nc.vector.reciprocal(stmp[:], stmp[:])
        nc.vector.tensor_scalar(sh_all[:, cs], stmp[:], 127.0, None, OP.mult)
        rh = scal.tile([P, 1], F32, tag="scal", name="rh")
        nc.vector.reciprocal(rh[:], sh_all[:, cs])
        nc.vector.tensor_tensor(de_all[:, cs], rh[:], c2, OP.mult)

    def emit_hq(tt):
        tsl = slice(tt * P, (tt + 1) * P)
        for gq in range(NGC):
            gt = stage.tile([P, GC], F32, tag="stage", name=f"gt{gq}")
            nc.sync.dma_start(gt[:], g_dram[tsl, gq * GC:(gq + 1) * GC])
            nc.vector.tensor_scalar(gt[:], gt[:], sh_all[:, tt:tt + 1],
                                    MAGIC, OP.mult, OP.add)
            hqn = qb.tile([P, GC], BF16, tag="qb")
            nc.vector.tensor_scalar(hqn[:], gt[:], MAGIC, None, OP.subtract)
            nc.sync.dma_start_transpose(
                hqT[:, gq * (GC // P):(gq + 1) * (GC // P), tsl], hqn[:])

    # ---------------- phase 1: mm1/mm3 paired, h = silu(h1)*h3 --------
    pending = {0: build_w13(0), 1: build_w13(1)}
    for hb in range(NHB):
        if hb == NHB - 1:
            # two w2 pass-A chunk tiles recycle the hb-1 wT ring slots
            w2A = [build_w2_tile(0, 0), build_w2_tile(1, 0)]
        wTs = pending.pop(hb)
        for tt in range(TT):
            tsl = slice(tt * P, (tt + 1) * P)
            psa = ps1.tile([P, HB], F32, tag="ps")
            psb = ps1.tile([P, HB], F32, tag="ps")
            for ko in range(KD):
                lhsT = xq[ko // 8][:, ko % 8, tsl]
                wt = wTs[ko // 8]
                r = ko % 8
                nc.tensor.matmul(psa[:], lhsT, wt[:, r, 0:HB],
                                 start=(ko == 0), stop=(ko == KD - 1))
                nc.tensor.matmul(psb[:], lhsT, wt[:, r, HB:2 * HB],
                                 start=(ko == 0), stop=(ko == KD - 1))
            sA = hwork.tile([P, HB], F32, tag="hw", name="sA")
            nc.scalar.activation(sA[:], psa[:], ACTF.Silu,
                                 scale=al_all[:, tt:tt + 1])
            h3 = hwork.tile([P, HB], F32, tag="hw", name="h3")
            nc.scalar.mul(h3[:], psb[:], be_all[:, tt:tt + 1])
            hh = hwork.tile([P, HB], F32, tag="hw", name="hh")
            nc.vector.tensor_tensor(hh[:], sA[:], h3[:], OP.mult)
            mpart = scal.tile([P, 1], F32, tag="scal", name="mpart")
            nc.vector.tensor_reduce(mpart[:], hh[:], AX.X, OP.max,
                                    apply_absolute_value=True)
            nc.vector.tensor_tensor(mh_all[:, tt:tt + 1],
                                    mh_all[:, tt:tt + 1], mpart[:], OP.max)
            nc.sync.dma_start(g_dram[tsl, hb * HB:(hb + 1) * HB], hh[:])
            if hb == NHB - 1:
                emit_sh(tt)
                emit_hq(tt)
        # build-ahead emitted after this block's matmuls so the freed
        # ring slots' WAR deps are visible to the allocator
        if hb + 2 < NHB:
            pending[hb + 2] = build_w13(hb + 2)

    # ---------------- phase 3: mm2 in two d_model column-pair passes --
    def phase3_pass(w2tiles, colbase):
        for ti in range(len(w2tiles), 6):
            w2tiles.append(build_w2_tile(ti, colbase))
        for tt in range(TT):
            tsl = slice(tt * P, (tt + 1) * P)
            psE = ps3.tile([P, 512], F32, tag="ps3")
            psO = ps3.tile([P, 512], F32, tag="ps3")
            for hc in range(KH):
                lhsT = hqT[:, hc, tsl]
                wt = w2tiles[hc // 8]
                r = hc % 8
                nc.tensor.matmul(psE[:], lhsT, wt[:, r, 0:512],
                                 start=(hc == 0), stop=(hc == KH - 1))
                nc.tensor.matmul(psO[:], lhsT, wt[:, r, 512:1024],
                                 start=(hc == 0), stop=(hc == KH - 1))
            for half, ps in ((0, psE), (1, psO)):
                ysb = yout.tile([P, 512], F32, tag="qb", name="ysb")
                nc.vector.tensor_scalar(ysb[:], ps[:],
                                        de_all[:, tt:tt + 1], None, OP.mult)
                nc.sync.dma_start(
                    y[tsl, colbase + half * 512:colbase + (half + 1) * 512],
                    ysb[:])

    phase3_pass(w2A, 0)
    phase3_pass([], 1024)

    for cm in reversed(ctxs):
        cm.__exit__(None, None, None)


_NC_CACHE = None


def _get_nc():
    global _NC_CACHE
    if _NC_CACHE is None:
        _NC_CACHE = _build()
    return _NC_CACHE


def kernel(x, w1, w2, w3, trace=False):
    x = np.ascontiguousarray(np.asarray(x, dtype=np.float32))
    w1 = np.asarray(w1, dtype=np.float32)
    w2 = np.asarray(w2, dtype=np.float32)
    w3 = np.asarray(w3, dtype=np.float32)
    w1t = np.ascontiguousarray(w1.T)
    w2t = np.ascontiguousarray(w2.T)
    w3t = np.ascontiguousarray(w3.T)
    B, S, Dm = x.shape
    xf = x.reshape(B * S, Dm)

    in_maps = []
    slc = (H * D) // NCORES
    for i in range(NCORES):
        wsl = np.stack([
            w1.reshape(-1)[i * slc:(i + 1) * slc].reshape(P, WSLF),
            w3.reshape(-1)[i * slc:(i + 1) * slc].reshape(P, WSLF),
            w2.reshape(-1)[i * slc:(i + 1) * slc].reshape(P, WSLF),
        ]).astype(np.float32)
        in_maps.append(dict(
            x=np.ascontiguousarray(xf[i * T:(i + 1) * T]),
            w1t=w1t, w2t=w2t, w3t=w3t, wsl=wsl))

    nc = _get_nc()
    res = bass_utils.run_bass_kernel_spmd(
        nc, in_maps, core_ids=list(range(NCORES)),
        trace=trace, trace_cores=[0] if trace else None)
    out = np.concatenate([res.results[i]["y"] for i in range(NCORES)], axis=0)
    if trace:
        kernel.last_results = res
    return out.reshape(B, S, Dm)
